# revision 1
# baseline (speedup 1.0000x reference)
"""LoRA MultiheadAttention on 8 Trainium2 NeuronCores (Bass/Tile).

Sharding: core c = (batch n = c//2, head-group hg = c%2); each core handles
6 of 12 heads for one of 4 batches. LoRA is folded into the projection
weights on the host (W_eff = W + scale * up @ down — mathematically
identical). Inputs are shipped pre-transposed (E-major) per shard. Each core
computes q^T/k^T (E-major), v (S-major, with a ones column per head for the
softmax denominator), full-softmax attention in fp16 with fp32 accumulation,
and a half-K out-projection partial. The host sums the two partials per
batch and adds the output bias (pure unshard glue).
"""
import numpy as np

import concourse.bass as bass
import concourse.tile as tile
from concourse import bacc, mybir
from concourse.bass_utils import run_bass_kernel_spmd

L, N, E, H, R = 2048, 4, 768, 12, 16
ALPHA = 16.0
LORA_SCALE = ALPHA / R
HD = E // H          # 64
HG = 2               # head groups (column-parallel dimension)
HPG = H // HG        # 6 heads per group
EG = E // HG         # 384 columns per group
NC_ = 8
F32 = mybir.dt.float32
F16 = mybir.dt.float16
SCALE = 1.0 / float(np.sqrt(HD))  # folded into exp's input scale

_CACHED = {}


def _build():
    nc = bacc.Bacc()
    # per-core external I/O (shapes are per-shard)
    xqT = nc.dram_tensor("xqT", [E, L], F32, kind="ExternalInput")
    xkT = nc.dram_tensor("xkT", [E, L], F32, kind="ExternalInput")
    xvT = nc.dram_tensor("xvT", [E, L], F32, kind="ExternalInput")
    wqT = nc.dram_tensor("wqT", [E, EG], F32, kind="ExternalInput")
    wkT = nc.dram_tensor("wkT", [E, EG], F32, kind="ExternalInput")
    wvT = nc.dram_tensor("wvT", [E, EG], F32, kind="ExternalInput")
    woT = nc.dram_tensor("woT", [EG, E], F32, kind="ExternalInput")
    bq = nc.dram_tensor("bq", [EG], F32, kind="ExternalInput")
    bk = nc.dram_tensor("bk", [EG], F32, kind="ExternalInput")
    bv = nc.dram_tensor("bv", [EG], F32, kind="ExternalInput")
    out = nc.dram_tensor("out", [E, L], F32, kind="ExternalOutput")

    KC = E // 128    # 6 contraction chunks
    EC = EG // 128   # 3 output chunks per projection
    LT = L // 128    # 16 l/s tiles
    VW = HPG * (HD + 1)  # 390: per-head 64 v cols + 1 ones col

    with tile.TileContext(nc) as tc:
        with (
            tc.tile_pool(name="stage", bufs=2) as stage,
            tc.tile_pool(name="big", bufs=16) as big,
            tc.tile_pool(name="persist", bufs=1) as persist,
            tc.tile_pool(name="small", bufs=4) as small,
            tc.tile_pool(name="outsb", bufs=3) as outsb_pool,
            tc.tile_pool(name="psum", bufs=1, space="PSUM") as psum,
        ):
            # ---- constants / weights ----
            w16 = {}
            for pname, wdram in (("q", wqT), ("k", wkT), ("v", wvT)):
                for j in range(KC):
                    w32 = stage.tile([128, 2048], F32, tag="stage", name="w32")
                    nc.sync.dma_start(w32[:, :EG], wdram[j * 128:(j + 1) * 128, :])
                    wt = persist.tile([128, EG], F16, name=f"w16_{pname}{j}")
                    nc.vector.tensor_copy(wt[:], w32[:, :EG])
                    w16[pname, j] = wt
            wo16 = []
            for j in range(EC):
                w32 = stage.tile([128, 2048], F32, tag="stage", name="w32")
                nc.sync.dma_start(w32[:, :E], woT[j * 128:(j + 1) * 128, :])
                wt = persist.tile([128, E], F16, name=f"wo16_{j}")
                nc.vector.tensor_copy(wt[:], w32[:, :E])
                wo16.append(wt)

            bias_t = {}
            for bname, bdram in (("q", bq), ("k", bk)):
                for j in range(EC):
                    bt = persist.tile([128, 1], F32, name=f"b_{bname}{j}")
                    nc.sync.dma_start(bt[:], bdram[j * 128:(j + 1) * 128])
                    bias_t[bname, j] = bt

            # ---- projections ----
            qkT = {}   # ("q"|"k", e-chunk) -> (128, L) f16, E-major
            v_aug = []  # 16 tiles (128, VW) f16, per-head [64 v | 1.0]
            for pname, xdram in (("q", xqT), ("k", xkT), ("v", xvT)):
                x16 = []
                for j in range(KC):
                    x32 = stage.tile([128, 2048], F32, tag="stage", name="x32")
                    nc.sync.dma_start(x32[:], xdram[j * 128:(j + 1) * 128, :])
                    xt = big.tile([128, L], F16, tag="big", name="x16")
                    nc.vector.tensor_copy(xt[:], x32[:])
                    x16.append(xt)
                if pname in ("q", "k"):
                    for e in range(EC):
                        dst = persist.tile([128, L], F16, name=f"{pname}T{e}")
                        qkT[pname, e] = dst
                        for lc in range(2):
                            mm = psum.tile([128, 1024], F32, tag="mm", bufs=3,
                                           name="mm_proj")
                            for half in range(2):
                                o_sl = mm[:, half * 512:(half + 1) * 512]
                                l0 = lc * 1024 + half * 512
                                for kk in range(KC):
                                    nc.tensor.matmul(
                                        o_sl,
                                        w16[pname, kk][:, e * 128:(e + 1) * 128],
                                        x16[kk][:, l0:l0 + 512],
                                        start=(kk == 0), stop=(kk == KC - 1),
                                    )
                            nc.vector.tensor_scalar_add(
                                dst[:, lc * 1024:(lc + 1) * 1024], mm[:],
                                bias_t[pname, e][:],
                            )
                else:
                    for st in range(LT):
                        mm = psum.tile([128, 1024], F32, tag="mm", bufs=3,
                                       name="mm_vproj")
                        for kk in range(KC):
                            nc.tensor.matmul(
                                mm[:, 0:EG],
                                x16[kk][:, st * 128:(st + 1) * 128],
                                w16["v", kk][:],
                                start=(kk == 0), stop=(kk == KC - 1),
                            )
                        vt = persist.tile([128, VW], F16, name=f"v_aug{st}")
                        grp = vt.rearrange("p (h c) -> p h c", c=HD + 1)
                        nc.vector.tensor_copy(
                            grp[:, :, 0:HD],
                            mm[:, 0:EG].rearrange("p (h c) -> p h c", c=HD),
                        )
                        nc.vector.memset(grp[:, :, HD:HD + 1], 1.0)
                        v_aug.append(vt)

            # ---- attention (v-stationary attnV: o^T produced directly) ----
            oT = [persist.tile([128, L], F16, name=f"oT{j}") for j in range(EC)]
            for h in range(HPG):
                et, pb = h // 2, (h % 2) * 64
                qs = qkT["q", et][pb:pb + 64, :]
                ks = qkT["k", et][pb:pb + 64, :]
                attn = []
                for st in range(LT):
                    at = big.tile([128, L], F16, tag="big", name="attn")
                    for lc in range(2):
                        sc = psum.tile([128, 1024], F32, tag="mm", bufs=3,
                                       name="mm_sc")
                        for half in range(2):
                            l0 = lc * 1024 + half * 512
                            nc.tensor.matmul(
                                sc[:, half * 512:(half + 1) * 512],
                                ks[:, st * 128:(st + 1) * 128],
                                qs[:, l0:l0 + 512],
                                start=True, stop=True,
                            )
                        nc.scalar.activation(
                            at[:, lc * 1024:(lc + 1) * 1024], sc[:],
                            mybir.ActivationFunctionType.Exp, scale=SCALE,
                        )
                    attn.append(at)
                # o^T_raw (65, L): rows 0-63 = head output (E-major), row 64
                # = softmax denominator (from the ones column of v_aug)
                oTh = persist.tile([65, L], F16, name="oTh", tag="oTh", bufs=2)
                for lc in range(4):
                    ot = psum.tile([65, 512], F32, tag="ot", bufs=2, name="ot")
                    for st in range(LT):
                        nc.tensor.matmul(
                            ot[:],
                            v_aug[st][:, h * (HD + 1):(h + 1) * (HD + 1)],
                            attn[st][:, lc * 512:(lc + 1) * 512],
                            start=(st == 0), stop=(st == LT - 1),
                        )
                    nc.vector.tensor_copy(
                        oTh[:, lc * 512:(lc + 1) * 512], ot[:])
                rec = small.tile([1, L], F16, tag="rec", bufs=2, name="rec")
                with nc.allow_low_precision("softmax denom reciprocal in f16"):
                    nc.vector.reciprocal(rec[:], oTh[64:65, :])
                rbc = small.tile([64, L], F16, tag="rbc", bufs=2, name="rbc")
                nc.gpsimd.partition_broadcast(rbc[:], rec[:])
                nc.vector.tensor_mul(
                    oT[et][pb:pb + 64, :], oTh[0:64, :], rbc[:])

            # ---- out-projection (out^T = W_o^T-chunks @ o^T) ----
            for lc in range(4):
                for eo in range(6):
                    po = psum.tile([128, 1024], F32, tag="mm", bufs=3,
                                   name="mm_out")
                    for j in range(EC):
                        nc.tensor.matmul(
                            po[:, 0:512],
                            wo16[j][:, eo * 128:(eo + 1) * 128],
                            oT[j][:, lc * 512:(lc + 1) * 512],
                            start=(j == 0), stop=(j == EC - 1),
                        )
                    osb = outsb_pool.tile([128, 512], F32, tag="osb", bufs=4,
                                          name="osb")
                    nc.vector.tensor_copy(osb[:], po[:, 0:512])
                    nc.sync.dma_start(
                        out[eo * 128:(eo + 1) * 128,
                            lc * 512:(lc + 1) * 512], osb[:])
    nc.finalize()
    return nc


def kernel(query, key, value, in_proj_weight, in_proj_bias,
           q_down, q_up, k_down, k_up, v_down, v_up,
           out_proj_weight, out_proj_bias, out_down, out_up):
    if "nc" not in _CACHED:
        _CACHED["nc"] = _build()
    nc = _CACHED["nc"]

    f = np.float32
    # fold LoRA into the projection weights (exact algebraic identity)
    w_eff = {}
    for i, (dn, up) in enumerate(((q_down, q_up), (k_down, k_up),
                                  (v_down, v_up))):
        w = in_proj_weight[i * E:(i + 1) * E].astype(f)
        w_eff[i] = w + LORA_SCALE * (up.astype(f) @ dn.astype(f))
    wo_eff = out_proj_weight.astype(f) + LORA_SCALE * (
        out_up.astype(f) @ out_down.astype(f))

    in_maps = []
    for c in range(NC_):
        n, hg = c // 2, c % 2
        sl = slice(hg * EG, (hg + 1) * EG)
        m = {
            "xqT": np.ascontiguousarray(query[:, n, :].T, dtype=f),
            "xkT": np.ascontiguousarray(key[:, n, :].T, dtype=f),
            "xvT": np.ascontiguousarray(value[:, n, :].T, dtype=f),
            "wqT": np.ascontiguousarray(w_eff[0][sl].T, dtype=f),
            "wkT": np.ascontiguousarray(w_eff[1][sl].T, dtype=f),
            "wvT": np.ascontiguousarray(w_eff[2][sl].T, dtype=f),
            "woT": np.ascontiguousarray(wo_eff[:, sl].T, dtype=f),
            "bq": np.ascontiguousarray(in_proj_bias[0:E][sl], dtype=f),
            "bk": np.ascontiguousarray(in_proj_bias[E:2 * E][sl], dtype=f),
            "bv": np.ascontiguousarray(in_proj_bias[2 * E:3 * E][sl], dtype=f),
        }
        in_maps.append(m)

    _CACHED["in_maps"] = in_maps
    res = run_bass_kernel_spmd(nc, in_maps, list(range(NC_)))
    outp = np.empty((L, N, E), dtype=np.float32)
    bo_total = out_proj_bias.astype(f) + wo_eff @ np.ascontiguousarray(
        in_proj_bias[2 * E:3 * E], dtype=f)
    for n in range(N):
        outp[:, n, :] = (res.results[2 * n]["out"]
                         + res.results[2 * n + 1]["out"]).T + bo_total
    return outp



# revision 12
# speedup vs baseline: 1.1315x; 1.1315x over previous
"""LoRA MultiheadAttention on 8 Trainium2 NeuronCores (Bass/Tile) — v2.

Sharding: core c = (batch n = c//2, head-group hg = c%2); each core handles
6 of 12 heads for one of 4 batches. LoRA is folded into the projection
weights on the host (W_eff = W + scale * up @ down — exact identity).

v2 changes vs v1 (436µs):
- Activations/weights shipped f16 from host: removes the on-chip f32->f16
  CAST pass (55µs DVE) and halves input DMA.
- Softmax reciprocal via reciprocal_approx_fast on a [2, L] pair-packed
  f32 tile (v1: 78µs of single-lane [1, L] RECIPROCAL) and the per-head
  row broadcast via a K=2 matmul against a constant selection matrix
  (v1: 19µs gpsimd partition_broadcast).
- Software pipelining: head h-1's attnV chains and the pair
  normalizations are interleaved into head h's scores/exp emission so
  TensorE works through ScalarE-bound stretches instead of queueing
  behind it.
- One shared [128, 2048] f16 ring holds both the input chunks and the
  attention tiles, so x staging is recycled into attn storage.
- f16 output partials (host sums in f32 and adds the folded bias).
"""
import numpy as np

import concourse.bass as bass
import concourse.tile as tile
from concourse import bacc, mybir
from concourse.bass_utils import run_bass_kernel_spmd

L, N, E, H, R = 2048, 4, 768, 12, 16
ALPHA = 16.0
LORA_SCALE = ALPHA / R
HD = E // H          # 64
HG = 2               # head groups (column-parallel dimension)
HPG = H // HG        # 6 heads per group
EG = E // HG         # 384 columns per group
NC_ = 8
F32 = mybir.dt.float32
F16 = mybir.dt.float16
SCALE = 1.0 / float(np.sqrt(HD))  # folded into exp's input scale

KC = E // 128    # 6 contraction chunks
EC = EG // 128   # 3 output chunks per projection (= n head pairs)
LT = L // 128    # 16 s tiles
VW = HPG * (HD + 1)  # 390: per-head 64 v cols + 1 ones col

_CACHED = {}


def _build():
    nc = bacc.Bacc()
    xqT = nc.dram_tensor("xqT", [E, L], F16, kind="ExternalInput")
    xkT = nc.dram_tensor("xkT", [E, L], F16, kind="ExternalInput")
    xvT = nc.dram_tensor("xvT", [E, L], F16, kind="ExternalInput")
    wqT = nc.dram_tensor("wqT", [E, EG], F16, kind="ExternalInput")
    wkT = nc.dram_tensor("wkT", [E, EG], F16, kind="ExternalInput")
    wvT = nc.dram_tensor("wvT", [E, EG], F16, kind="ExternalInput")
    woT = nc.dram_tensor("woT", [EG, E], F16, kind="ExternalInput")
    bq = nc.dram_tensor("bq", [EG], F32, kind="ExternalInput")
    bk = nc.dram_tensor("bk", [EG], F32, kind="ExternalInput")
    out = nc.dram_tensor("out", [E, L], F16, kind="ExternalOutput")

    with tile.TileContext(nc) as tc:
        with (
            tc.tile_pool(name="big", bufs=24) as big,
            tc.tile_pool(name="persist", bufs=1) as persist,
            tc.tile_pool(name="small", bufs=1) as small,
            tc.tile_pool(name="outsb", bufs=4) as outsb_pool,
            tc.tile_pool(name="psum", bufs=1, space="PSUM") as psum,
        ):
            # ---- weights / constants (straight f16 DMA) ----
            w16 = {}
            for pname, wdram in (("q", wqT), ("k", wkT), ("v", wvT)):
                for j in range(KC):
                    wt = persist.tile([128, EG], F16, name=f"w16_{pname}{j}")
                    nc.sync.dma_start(wt[:], wdram[j * 128:(j + 1) * 128, :])
                    w16[pname, j] = wt
            wo16 = []
            for j in range(EC):
                wt = persist.tile([128, E], F16, name=f"wo16_{j}")
                nc.sync.dma_start(wt[:], woT[j * 128:(j + 1) * 128, :])
                wo16.append(wt)
            bias_t = {}
            for bname, bdram in (("q", bq), ("k", bk)):
                for j in range(EC):
                    bt = persist.tile([128, 1], F32, name=f"b_{bname}{j}")
                    nc.sync.dma_start(bt[:], bdram[j * 128:(j + 1) * 128])
                    bias_t[bname, j] = bt
            # constant selection matrix for the denominator broadcast matmul:
            # a single K=64 matmul with mask rows at partitions 0/32 (the
            # packed denominator rows) and zeros elsewhere spreads
            # rec(even head) to out partitions 0-63 and rec(odd) to 64-127.
            esel = persist.tile([64, 128], F16, name="esel")
            nc.vector.memset(esel[:], 0.0)
            nc.vector.memset(esel[0:1, 0:64], 1.0)
            nc.vector.memset(esel[32:33, 64:128], 1.0)

            qkT = {}    # ("q"|"k", e-chunk) -> (128, L) f16, E-major
            v_aug = []  # LT tiles (128, VW) f16, per-head [64 v | 1.0]
            oT = [persist.tile([128, L], F16, name=f"oT{j}")
                  for j in range(EC)]
            d2 = {p: small.tile([64, L], F32, name=f"d2_{p}")
                  for p in range(EC)}
            for p in range(EC):
                nc.vector.memset(d2[p][:], 1.0)
            attn_tiles = {}   # (head, st) -> [128, L] f16 ring tile

            def load_x(pname, xdram):
                xs = []
                for j in range(KC):
                    xt = big.tile([128, L], F16, tag="big", name="x16")
                    nc.sync.dma_start(xt[:], xdram[j * 128:(j + 1) * 128, :])
                    xs.append(xt)
                return xs

            def proj_qk(pname, xs, e):
                dst = persist.tile([128, L], F16, name=f"{pname}T{e}")
                qkT[pname, e] = dst
                for lc in range(2):
                    mm = psum.tile([128, 1024], F32, tag="sc", bufs=3,
                                   name="mm_proj")
                    for half in range(2):
                        o_sl = mm[:, half * 512:(half + 1) * 512]
                        l0 = lc * 1024 + half * 512
                        for kk in range(KC):
                            nc.tensor.matmul(
                                o_sl,
                                w16[pname, kk][:, e * 128:(e + 1) * 128],
                                xs[kk][:, l0:l0 + 512],
                                start=(kk == 0), stop=(kk == KC - 1),
                            )
                    nc.vector.tensor_scalar_add(
                        dst[:, lc * 1024:(lc + 1) * 1024], mm[:],
                        bias_t[pname, e][:],
                    )

            def proj_v(xs, st):
                mm = psum.tile([128, 1024], F32, tag="sc", bufs=3,
                               name="mm_vproj")
                for kk in range(KC):
                    nc.tensor.matmul(
                        mm[:, 0:EG],
                        xs[kk][:, st * 128:(st + 1) * 128],
                        w16["v", kk][:],
                        start=(kk == 0), stop=(kk == KC - 1),
                    )
                vt = persist.tile([128, VW], F16, name=f"v_aug{st}")
                grp = vt.rearrange("p (h c) -> p h c", c=HD + 1)
                nc.vector.tensor_copy(
                    grp[:, :, 0:HD],
                    mm[:, 0:EG].rearrange("p (h c) -> p h c", c=HD),
                )
                nc.vector.memset(grp[:, :, HD:HD + 1], 1.0)
                v_aug.append(vt)

            def scores_exp(h, background):
                """Scores+exp for head h, pulling interleaved background
                (previous head's attnV / pair norm) between chunks."""
                et, pb = h // 2, (h % 2) * 64
                qs = qkT["q", et][pb:pb + 64, :]
                ks = qkT["k", et][pb:pb + 64, :]
                for st in range(LT):
                    at = big.tile([128, L], F16, tag="big", name="attn")
                    attn_tiles[h, st] = at
                    for lc in range(2):
                        sc = psum.tile([128, 1024], F32, tag="sc", bufs=3,
                                       name="mm_sc")
                        for half in range(2):
                            l0 = lc * 1024 + half * 512
                            nc.tensor.matmul(
                                sc[:, half * 512:(half + 1) * 512],
                                ks[:, st * 128:(st + 1) * 128],
                                qs[:, l0:l0 + 512],
                                start=True, stop=True,
                            )
                        nc.scalar.activation(
                            at[:, lc * 1024:(lc + 1) * 1024], sc[:],
                            mybir.ActivationFunctionType.Exp, scale=SCALE,
                        )
                        for _ in range(3):
                            next(background, None)

            def attnv(h):
                """attnV chains for head h; oT rows + denominator row."""
                et, j = h // 2, h % 2
                for lc in range(4):
                    ot = psum.tile([65, 512], F32, tag="b512", bufs=2,
                                   name="ot")
                    for st in range(LT):
                        nc.tensor.matmul(
                            ot[:],
                            v_aug[st][:, h * (HD + 1):(h + 1) * (HD + 1)],
                            attn_tiles[h, st][:, lc * 512:(lc + 1) * 512],
                            start=(st == 0), stop=(st == LT - 1),
                        )
                        yield
                    nc.vector.tensor_copy(
                        oT[et][j * 64:(j + 1) * 64,
                               lc * 512:(lc + 1) * 512],
                        ot[0:64, :])
                    nc.vector.tensor_copy(
                        d2[et][32 * j:32 * j + 1, lc * 512:(lc + 1) * 512],
                        ot[64:65, :])
                    yield

            def norm(p):
                """Normalize pair p's oT rows by softmax denominators."""
                rec32 = small.tile([64, L], F32, tag="rec32", bufs=1,
                                   name="rec32")
                rec2 = small.tile([64, L], F16, tag="rec2", bufs=1,
                                  name="rec2")
                nc.vector.reciprocal_approx_fast(rec32[:], d2[p][:])
                nc.vector.tensor_copy(rec2[:], rec32[:])
                yield
                for lq in range(4):
                    bc = psum.tile([128, 512], F32, tag="b512", bufs=2,
                                   name="bc")
                    ls = slice(lq * 512, (lq + 1) * 512)
                    nc.tensor.matmul(
                        bc[:], esel[:], rec2[:, ls],
                        start=True, stop=True,
                    )
                    nc.vector.tensor_mul(
                        oT[p][:, ls],
                        oT[p][:, ls],
                        bc[:])
                    yield

            def chain(*gens):
                for g in gens:
                    yield from g

            # ---- emission schedule ----
            xq = load_x("q", xqT)
            xk = load_x("k", xkT)
            for e in range(EC):
                proj_qk("q", xq, e)
                proj_qk("k", xk, e)
            xv = load_x("v", xvT)
            for st in range(LT):
                proj_v(xv, st)

            empty = iter(())
            backgrounds = [
                empty,                             # during head 0
                attnv(0),                          # during head 1
                chain(attnv(1), norm(0)),          # during head 2
                attnv(2),                          # during head 3
                chain(attnv(3), norm(1)),          # during head 4
                attnv(4),                          # during head 5
            ]
            for h in range(HPG):
                scores_exp(h, backgrounds[h])
            for _ in chain(attnv(5), norm(2)):
                pass

            # ---- out-projection (out^T = W_o^T-chunks @ o^T) ----
            for lc in range(4):
                for eo in range(6):
                    po = psum.tile([128, 512], F32, tag="b512", bufs=2,
                                   name="mm_out")
                    for j in range(EC):
                        nc.tensor.matmul(
                            po[:],
                            wo16[j][:, eo * 128:(eo + 1) * 128],
                            oT[j][:, lc * 512:(lc + 1) * 512],
                            start=(j == 0), stop=(j == EC - 1),
                        )
                    osb = outsb_pool.tile([128, 512], F16, tag="osb", bufs=4,
                                          name="osb")
                    nc.vector.tensor_copy(osb[:], po[:])
                    nc.sync.dma_start(
                        out[eo * 128:(eo + 1) * 128,
                            lc * 512:(lc + 1) * 512], osb[:])
    nc.finalize()
    return nc


def kernel(query, key, value, in_proj_weight, in_proj_bias,
           q_down, q_up, k_down, k_up, v_down, v_up,
           out_proj_weight, out_proj_bias, out_down, out_up):
    if "nc" not in _CACHED:
        _CACHED["nc"] = _build()
    nc = _CACHED["nc"]

    f = np.float32
    h = np.float16
    # fold LoRA into the projection weights (exact algebraic identity)
    w_eff = {}
    for i, (dn, up) in enumerate(((q_down, q_up), (k_down, k_up),
                                  (v_down, v_up))):
        w = in_proj_weight[i * E:(i + 1) * E].astype(f)
        w_eff[i] = w + LORA_SCALE * (up.astype(f) @ dn.astype(f))
    wo_eff = out_proj_weight.astype(f) + LORA_SCALE * (
        out_up.astype(f) @ out_down.astype(f))

    in_maps = []
    for c in range(NC_):
        n, hg = c // 2, c % 2
        sl = slice(hg * EG, (hg + 1) * EG)
        m = {
            "xqT": np.ascontiguousarray(query[:, n, :].T, dtype=h),
            "xkT": np.ascontiguousarray(key[:, n, :].T, dtype=h),
            "xvT": np.ascontiguousarray(value[:, n, :].T, dtype=h),
            "wqT": np.ascontiguousarray(w_eff[0][sl].T, dtype=h),
            "wkT": np.ascontiguousarray(w_eff[1][sl].T, dtype=h),
            "wvT": np.ascontiguousarray(w_eff[2][sl].T, dtype=h),
            "woT": np.ascontiguousarray(wo_eff[:, sl].T, dtype=h),
            "bq": np.ascontiguousarray(in_proj_bias[0:E][sl], dtype=f),
            "bk": np.ascontiguousarray(in_proj_bias[E:2 * E][sl], dtype=f),
        }
        in_maps.append(m)

    _CACHED["in_maps"] = in_maps
    res = run_bass_kernel_spmd(nc, in_maps, list(range(NC_)))
    outp = np.empty((L, N, E), dtype=np.float32)
    bo_total = out_proj_bias.astype(f) + wo_eff @ np.ascontiguousarray(
        in_proj_bias[2 * E:3 * E], dtype=f)
    for n in range(N):
        outp[:, n, :] = (res.results[2 * n]["out"].astype(f)
                         + res.results[2 * n + 1]["out"].astype(f)).T + bo_total
    return outp
